# revision 53
# baseline (speedup 1.0000x reference)
"""Non-local (dot-product attention) block kernel for Trainium2, 8 cores.

Reference math (per sample):
    t = theta_w @ X + bt 1^T            (D, N)
    p = phi_w @ X + bp 1^T              (D, N)
    g_x = (g_w @ X + bg 1^T)^T          (N, D)
    f = t^T p / N;  y = f g_x;  z = BN(w_w y^T) + x

Gram-form collapse (no softmax => pure matmul associativity). Since phi/g
only appear through M = p g, and t only through W2 = V theta_w:
    S  = X X^T                      (C, C)  Gram matrix
    sx = X 1                        (C,)    row sums (free via ones column)
    Mt = g_w S phi_w^T + bg(u+N bp)^T + v bp^T   u=phi_w sx, v=g_w sx
    R1 = Mt^T w''^T  (= V^T)        w'' = diag(inv) w_w / N
    T2 = theta_w^T R1 + I           (= W2^T + I: residual folded into weights)
    b2 = R1^T bt + b'
    z  = T2^T X + b2 1^T            one (C,C) x (C,N) output matmul

So the per-pixel work is just two C x C x N matmuls (Gram + output); the
whole projection/bias machinery shrinks to a ~10-matmul D-sized chain.

Device plan per core (data-parallel, one sample per core, no collectives):
  - Inputs: x uploaded twice (n-major xt for the Gram, c-major xc for the
    output matmul), fp16; weights byte-packed into wpk + a 1-partition aux.
  - S accumulates in PSUM over 24 pixel chunks streamed by DMA; a host-side
    ones column (257th) makes column 256 of S the row-sums sx.
  - Chain runs on tiny matmuls (rank-1 bias terms are K=1 matmuls over
    host-packed rows) with ACT/DVE splitting the PSUM->SBUF hops.
  - Output phase: 12 (128,512) matmul pairs; per-partition bias b2 applied
    during the PSUM->SBUF copy (ACT half 0 / DVE half 1); 6 strided bf16
    out-DMAs on the sync ring; host widens to fp32.
"""

import numpy as np

B, C, HH, WW = 8, 256, 96, 32
N = HH * WW          # 3072
D = 128              # inter_channels
BN_EPS = 1e-5
NT = N // 128        # 24 pixel chunks
N_CORES = 8
NWARM = 4            # dummy matmuls to lift the PE HAM clock gate

_NC = None


def _build_nc():
    from contextlib import ExitStack

    import concourse.bass as bass
    import concourse.bacc as bacc
    import concourse.tile as tile
    from concourse import mybir

    f32 = mybir.dt.float32
    f16 = mybir.dt.bfloat16
    AF = mybir.ActivationFunctionType

    nc = bacc.Bacc(
        "TRN2",
        target_bir_lowering=False,
        debug=False,
        num_devices=N_CORES,
    )

    # xt: 24 blocks of 257 cols: block i = x^T[128i:128(i+1), :] | ones
    xt = nc.dram_tensor("xt", [128, NT * 257], f16, kind="ExternalInput").ap()
    # xc: 6 blocks of 1024: block j = [x[0:128, 512j:512j+512] | x[128:256, ...]]
    xc = nc.dram_tensor("xc", [128, 6144], f16, kind="ExternalInput").ap()
    # wpk f16 cols: pgWT 512 | wT2 256 | thW 256 | Ipk 512 | bt 2 | pad 2
    wpk = nc.dram_tensor("wpk", [128, 770], f32, kind="ExternalInput").ap()
    # aux f16 cols: bg 128 | bp 128 | b'0 128 | b'1 128 ; f32 cols 256:384 = N*bp
    aux = nc.dram_tensor("aux", [1, 384], f32, kind="ExternalInput").ap()
    # out: [z[0:128, :] | z[128:256, :]] fp16
    out = nc.dram_tensor("out", [128, 6144], f16, kind="ExternalOutput").ap()

    with tile.TileContext(nc) as tc, ExitStack() as ctx:
        const = ctx.enter_context(tc.tile_pool(name="const", bufs=1))
        zpool = ctx.enter_context(tc.tile_pool(name="zpool", bufs=4))
        psS = ctx.enter_context(tc.tile_pool(name="psS", bufs=1, space="PSUM"))
        psC = ctx.enter_context(tc.tile_pool(name="psC", bufs=2, space="PSUM"))
        psZ = ctx.enter_context(tc.tile_pool(name="psZ", bufs=3, space="PSUM"))

        xt_sb = const.tile([128, NT * 257], f16)
        xc_sb = const.tile([128, 6144], f16)
        wpk_sb = const.tile([128, 770], f32)
        aux_sb = const.tile([1, 384], f32)
        S0_sb = const.tile([128, 257], f16)
        S1_sb = const.tile([128, 257], f16)
        uv_sb = const.tile([1, 256], f16)
        SG_sb = const.tile([128, 256], f16)
        Mt_sb = const.tile([128, 128], f16)
        R1_sb = const.tile([128, 256], f16)
        T2_sb = const.tile([128, 512], f16)
        b2_sb = const.tile([128, 2], f32)

        wpk16 = wpk_sb.bitcast(f16)   # (128, 1540)
        aux16 = aux_sb.bitcast(f16)   # (1, 768)

        # Each dma_start costs ~610ns on its issuing sequencer and rings
        # process their FIFO in order, so: xt pieces own the sync ring (the
        # S phase streams them; first piece small so S starts early), xc
        # follows on the same ring (needed only at the output phase),
        # wpk/aux ride the scalar ring.
        nc.scalar.dma_start(out=wpk_sb, in_=wpk)
        nc.scalar.dma_start(out=aux_sb, in_=aux)
        piece_chunks = [2, 2, 4, 4, 6, 6]
        pc0 = 0
        for pc in piece_chunks:
            psl = slice(pc0 * 257, (pc0 + pc) * 257)
            nc.sync.dma_start(out=xt_sb[:, psl], in_=xt[:, psl])
            pc0 += pc
        for q in range(2):
            qsl = slice(q * 3072, (q + 1) * 3072)
            nc.sync.dma_start(out=xc_sb[:, qsl], in_=xc[:, qsl])

        # S = X X^T accumulated over 24 pixel chunks; col 256 = sx (ones col)
        S0 = psS.tile([128, 512], f32, tag="s0", name="S0")[:, 0:257]
        S1 = psS.tile([128, 512], f32, tag="s1", name="S1")[:, 0:257]
        for i in range(NT):
            base = i * 257
            nc.tensor.matmul(
                S0, lhsT=xt_sb[:, base : base + 128],
                rhs=xt_sb[:, base : base + 257],
                start=(i == 0), stop=(i == NT - 1),
            )
            nc.tensor.matmul(
                S1, lhsT=xt_sb[:, base + 128 : base + 256],
                rhs=xt_sb[:, base : base + 257],
                start=(i == 0), stop=(i == NT - 1),
            )
        nc.scalar.copy(out=S0_sb, in_=S0)
        nc.vector.tensor_copy(S1_sb, S1)

        # SG[c, d'] = sum_c2 S[c2, c] g_w[d', c2]  (S symmetric) -- first on
        # the PE: it heads the critical chain S -> SG -> Mt -> R1 -> T2.
        SGp = psC.tile([128, 256], f32, tag="c", name="SGp")
        for h in range(2):
            hsl = slice(h * 128, (h + 1) * 128)
            nc.tensor.matmul(
                SGp[:, hsl], lhsT=S0_sb[:, hsl], rhs=wpk16[:, 128:256],
                start=True, stop=False,
            )
            nc.tensor.matmul(
                SGp[:, hsl], lhsT=S1_sb[:, hsl], rhs=wpk16[:, 384:512],
                start=False, stop=True,
            )
        # uv = [u | v] = sx^T [phi_w^T | g_w^T]
        uvp = psC.tile([128, 256], f32, tag="c", name="uvp")
        nc.tensor.matmul(
            uvp[0:1, :], lhsT=S0_sb[:, 256:257], rhs=wpk16[:, 0:256],
            start=True, stop=False,
        )
        nc.tensor.matmul(
            uvp[0:1, :], lhsT=S1_sb[:, 256:257], rhs=wpk16[:, 256:512],
            start=False, stop=True,
        )
        nc.scalar.copy(out=SG_sb[:, 0:128], in_=SGp[:, 0:128])
        nc.vector.tensor_copy(SG_sb[:, 128:256], SGp[:, 128:256])
        nc.vector.tensor_copy(uv_sb, uvp[0:1, :])

        # Mt[d', d] = M[d, d'] = SG^T phi_w^T + bg u^T + v bp^T + N bg bp^T
        # (all three rank-1 terms are pure matmuls on host-packed rows)
        Mtp = psC.tile([128, 128], f32, tag="c", name="Mtp")
        nc.tensor.matmul(
            Mtp, lhsT=SG_sb[:, 0:128], rhs=wpk16[:, 0:128],
            start=True, stop=False,
        )
        nc.tensor.matmul(
            Mtp, lhsT=SG_sb[:, 128:256], rhs=wpk16[:, 256:384],
            start=False, stop=False,
        )
        nc.tensor.matmul(
            Mtp, lhsT=aux16[0:1, 0:128], rhs=uv_sb[0:1, 0:128],
            start=False, stop=False,
        )
        nc.tensor.matmul(
            Mtp, lhsT=uv_sb[0:1, 128:256], rhs=aux16[0:1, 128:256],
            start=False, stop=False,
        )
        nc.tensor.matmul(
            Mtp, lhsT=aux16[0:1, 0:128], rhs=aux16[0:1, 512:640],
            start=False, stop=True,
        )
        nc.scalar.copy(out=Mt_sb[:, 0:64], in_=Mtp[:, 0:64])
        nc.vector.tensor_copy(Mt_sb[:, 64:128], Mtp[:, 64:128])

        # R1[d, c] = sum_d' Mt[d', d] w''[c, d']  (= V[c, d])
        R1p = psC.tile([128, 256], f32, tag="c", name="R1p")
        nc.tensor.matmul(R1p, lhsT=Mt_sb, rhs=wpk16[:, 512:768], start=True, stop=True)
        nc.scalar.copy(out=R1_sb[:, 0:128], in_=R1p[:, 0:128])
        nc.vector.tensor_copy(R1_sb[:, 128:256], R1p[:, 128:256])

        # T2 halves: T2[c', c] = theta_w^T R1 + I  (identity matmuls first:
        # they only need wpk + a free PSUM buffer, so they run early)
        T2ps = [
            psC.tile([128, 256], f32, tag="c", name=f"T2p{h}") for h in range(2)
        ]
        for h in range(2):
            nc.tensor.matmul(
                T2ps[h], lhsT=wpk16[:, 1024:1152],
                rhs=wpk16[:, 1024 + 256 * h : 1280 + 256 * h],
                start=True, stop=False,
            )
        for h in range(2):
            nc.tensor.matmul(
                T2ps[h], lhsT=wpk16[:, 768 + 128 * h : 896 + 128 * h], rhs=R1_sb,
                start=False, stop=True,
            )
        nc.scalar.copy(out=T2_sb[:, 0:256], in_=T2ps[0])
        nc.vector.tensor_copy(T2_sb[:, 256:512], T2ps[1])

        # z[c, n] = sum_c' T2[c', c] X[c', n] + b2[c]; fp16 out.
        # Per 512-pixel chunk j: two PSUM tiles (one per c-half), assembled
        # in parallel by ACT (half 0, bias via activation) and DVE (half 1,
        # tensor_scalar), one sync-issued DMA covering both c-halves of the
        # chunk via a strided DRAM AP. b2's tiny matmuls are emitted after
        # chunk 0's so they don't delay the first output matmul.
        for j in range(6):
            z_sb = zpool.tile([128, 1024], f16, tag="zs", name=f"z{j}")
            pzs = []
            for hc in range(2):
                pz = psZ.tile([128, 512], f32, tag="z", name=f"pz{j}_{hc}")
                pzs.append(pz)
                nc.tensor.matmul(
                    pz, lhsT=T2_sb[:, 128 * hc : 128 * hc + 128],
                    rhs=xc_sb[:, j * 1024 : j * 1024 + 512],
                    start=True, stop=False,
                )
                nc.tensor.matmul(
                    pz, lhsT=T2_sb[:, 256 + 128 * hc : 384 + 128 * hc],
                    rhs=xc_sb[:, j * 1024 + 512 : (j + 1) * 1024],
                    start=False, stop=True,
                )
            if j == 0:
                # b2[c] = sum_d R1[d, c] bt[d] + b'[c]
                b2p = psC.tile([128, 2], f32, tag="c", name="b2p")
                for h in range(2):
                    nc.tensor.matmul(
                        b2p[:, h : h + 1],
                        lhsT=R1_sb[:, h * 128 : (h + 1) * 128],
                        rhs=wpk16[:, 1536:1537], start=True, stop=False,
                    )
                    nc.tensor.matmul(
                        b2p[:, h : h + 1],
                        lhsT=aux16[0:1, 256 + 128 * h : 384 + 128 * h],
                        rhs=wpk16[0:1, 1024:1025], start=False, stop=True,
                    )
                nc.vector.tensor_copy(b2_sb, b2p)
            nc.scalar.activation(
                out=z_sb[:, 0:512], in_=pzs[0], func=AF.Identity,
                bias=b2_sb[:, 0:1], scale=1.0,
            )
            nc.vector.tensor_scalar_add(z_sb[:, 512:1024], pzs[1], b2_sb[:, 1:2])
            out_ap = bass.AP(
                tensor=out.tensor, offset=j * 512,
                ap=[[6144, 128], [3072, 2], [1, 512]],
            )
            nc.sync.dma_start(out=out_ap, in_=z_sb)

    nc.compile()
    return nc


def _get_nc():
    global _NC
    if _NC is None:
        _NC = _build_nc()
    return _NC


# test.py reads this after a traced run to get exec_time_ns
last_results = None


def _prep_inputs(inputs):
    import ml_dtypes

    f16 = ml_dtypes.bfloat16

    x = np.asarray(inputs["x"], dtype=np.float32)
    theta_w = np.asarray(inputs["theta_w"], np.float32)
    theta_b = np.asarray(inputs["theta_b"], np.float32)
    phi_w = np.asarray(inputs["phi_w"], np.float32)
    phi_b = np.asarray(inputs["phi_b"], np.float32)
    g_w = np.asarray(inputs["g_w"], np.float32)
    g_b = np.asarray(inputs["g_b"], np.float32)
    w_w = np.asarray(inputs["w_w"], np.float32)
    w_b = np.asarray(inputs["w_b"], np.float32)
    bn_gamma = np.asarray(inputs["bn_gamma"], np.float32)
    bn_beta = np.asarray(inputs["bn_beta"], np.float32)
    bn_mean = np.asarray(inputs["bn_mean"], np.float32)
    bn_var = np.asarray(inputs["bn_var"], np.float32)

    inv = bn_gamma / np.sqrt(bn_var + BN_EPS)
    bprime = inv * (w_b - bn_mean) + bn_beta                  # (C,)
    wpp = (w_w * inv[:, None]) / N                            # w'' (C, D)

    # wpk: per-partition packed weights (f16 cols)
    pgWT = np.concatenate(
        [phi_w.T[0:128], g_w.T[0:128], phi_w.T[128:256], g_w.T[128:256]],
        axis=1,
    )                                                         # (128, 512)
    Ipk = np.zeros((128, 512), np.float32)
    Ipk[np.arange(128), np.arange(128)] = 1.0                 # [I | 0]
    Ipk[np.arange(128), 384 + np.arange(128)] = 1.0           # [0 | I]
    btc = np.zeros((128, 4), np.float32)
    btc[:, 0] = theta_b
    wpk_f16 = np.concatenate(
        [pgWT, wpp.T, theta_w, Ipk, btc], axis=1
    ).astype(f16)                                             # (128, 1540)
    assert wpk_f16.shape == (128, 1540), wpk_f16.shape
    wpk = np.ascontiguousarray(wpk_f16).view(np.uint8).view(np.float32)      # (128, 770)

    aux_f16 = np.concatenate(
        [g_b, phi_b, bprime, N * phi_b, np.zeros(128, np.float32)]
    ).astype(f16)                                             # 768 f16 = 1536 B
    aux = aux_f16.view(np.uint8).view(np.float32)[None, :]                   # (1, 384)

    x16 = x.reshape(B, C, N).astype(f16)
    xt = np.ones((B, NT, 128, 257), f16)
    xt[:, :, :, 0:256] = x16.transpose(0, 2, 1).reshape(B, NT, 128, C)
    xt = xt.reshape(B, 128 * NT, 257)  # will re-chunk below
    xt = np.ascontiguousarray(
        xt.reshape(B, NT, 128, 257).transpose(0, 2, 1, 3).reshape(B, 128, NT * 257)
    )
    xc = np.ascontiguousarray(
        x16.reshape(B, 2, 128, 6, 512).transpose(0, 2, 3, 1, 4).reshape(B, 128, 6144)
    )
    return xt, xc, {"wpk": wpk, "aux": aux}


def kernel(**inputs):
    from concourse.bass_utils import run_bass_kernel_spmd

    global last_results

    xt, xc, shared = _prep_inputs(inputs)
    in_maps = [
        dict(shared, xt=np.ascontiguousarray(xt[b]), xc=np.ascontiguousarray(xc[b]))
        for b in range(B)
    ]

    nc = _get_nc()
    res = run_bass_kernel_spmd(nc, in_maps, list(range(N_CORES)))
    last_results = res

    outs = np.stack([res.results[b]["out"] for b in range(B)])  # (B, 128, 6144)
    z = outs.reshape(B, 128, 2, 3072).transpose(0, 2, 1, 3).reshape(B, C, N)
    return z.reshape(B, C, HH, WW).astype(np.float32)


# revision 54
# speedup vs baseline: 1.0103x; 1.0103x over previous
"""Non-local (dot-product attention) block kernel for Trainium2, 8 cores.

Reference math (per sample):
    t = theta_w @ X + bt 1^T            (D, N)
    p = phi_w @ X + bp 1^T              (D, N)
    g_x = (g_w @ X + bg 1^T)^T          (N, D)
    f = t^T p / N;  y = f g_x;  z = BN(w_w y^T) + x

Gram-form collapse (no softmax => pure matmul associativity). Since phi/g
only appear through M = p g, and t only through W2 = V theta_w:
    S  = X X^T                      (C, C)  Gram matrix
    sx = X 1                        (C,)    row sums (free via ones column)
    Mt = g_w S phi_w^T + bg(u+N bp)^T + v bp^T   u=phi_w sx, v=g_w sx
    R1 = Mt^T w''^T  (= V^T)        w'' = diag(inv) w_w / N
    T2 = theta_w^T R1 + I           (= W2^T + I: residual folded into weights)
    b2 = R1^T bt + b'
    z  = T2^T X + b2 1^T            one (C,C) x (C,N) output matmul

So the per-pixel work is just two C x C x N matmuls (Gram + output); the
whole projection/bias machinery shrinks to a ~10-matmul D-sized chain.

Device plan per core (data-parallel, one sample per core, no collectives):
  - Inputs: x uploaded twice (n-major xt for the Gram, c-major xc for the
    output matmul), fp16; weights byte-packed into wpk + a 1-partition aux.
  - S accumulates in PSUM over 24 pixel chunks streamed by DMA; a host-side
    ones column (257th) makes column 256 of S the row-sums sx.
  - Chain runs on tiny matmuls (rank-1 bias terms are K=1 matmuls over
    host-packed rows) with ACT/DVE splitting the PSUM->SBUF hops.
  - Output phase: 12 (128,512) matmul pairs; per-partition bias b2 applied
    during the PSUM->SBUF copy (ACT half 0 / DVE half 1); 6 strided bf16
    out-DMAs on the sync ring; host widens to fp32.
"""

import numpy as np

B, C, HH, WW = 8, 256, 96, 32
N = HH * WW          # 3072
D = 128              # inter_channels
BN_EPS = 1e-5
NT = N // 128        # 24 pixel chunks
N_CORES = 8
NWARM = 4            # dummy matmuls to lift the PE HAM clock gate

_NC = None


def _build_nc():
    from contextlib import ExitStack

    import concourse.bass as bass
    import concourse.bacc as bacc
    import concourse.tile as tile
    from concourse import mybir

    f32 = mybir.dt.float32
    f16 = mybir.dt.bfloat16
    AF = mybir.ActivationFunctionType

    nc = bacc.Bacc(
        "TRN2",
        target_bir_lowering=False,
        debug=False,
        num_devices=N_CORES,
    )

    # xt: 24 blocks of 257 cols: block i = x^T[128i:128(i+1), :] | ones
    xt = nc.dram_tensor("xt", [128, NT * 257], f16, kind="ExternalInput").ap()
    # xc: 6 blocks of 1024: block j = [x[0:128, 512j:512j+512] | x[128:256, ...]]
    xc = nc.dram_tensor("xc", [128, 6144], f16, kind="ExternalInput").ap()
    # wpk f16 cols: pgWT 512 | wT2 256 | thW 256 | Ipk 512 | bt 2 | pad 2
    wpk = nc.dram_tensor("wpk", [128, 770], f32, kind="ExternalInput").ap()
    # aux f16 cols: bg 128 | bp 128 | b'0 128 | b'1 128 ; f32 cols 256:384 = N*bp
    aux = nc.dram_tensor("aux", [1, 384], f32, kind="ExternalInput").ap()
    # out: [z[0:128, :] | z[128:256, :]] fp16
    out = nc.dram_tensor("out", [128, 6144], f16, kind="ExternalOutput").ap()

    with tile.TileContext(nc) as tc, ExitStack() as ctx:
        const = ctx.enter_context(tc.tile_pool(name="const", bufs=1))
        zpool = ctx.enter_context(tc.tile_pool(name="zpool", bufs=4))
        psS = ctx.enter_context(tc.tile_pool(name="psS", bufs=1, space="PSUM"))
        psC = ctx.enter_context(tc.tile_pool(name="psC", bufs=2, space="PSUM"))
        psZ = ctx.enter_context(tc.tile_pool(name="psZ", bufs=3, space="PSUM"))

        xt_sb = const.tile([128, NT * 257], f16)
        xc_sb = const.tile([128, 6144], f16)
        wpk_sb = const.tile([128, 770], f32)
        aux_sb = const.tile([1, 384], f32)
        S0_sb = const.tile([128, 257], f16)
        S1_sb = const.tile([128, 257], f16)
        uv_sb = const.tile([1, 256], f16)
        SG_sb = const.tile([128, 256], f16)
        Mt_sb = const.tile([128, 128], f16)
        R1_sb = const.tile([128, 256], f16)
        T2_sb = const.tile([128, 512], f16)
        b2_sb = const.tile([128, 2], f32)

        wpk16 = wpk_sb.bitcast(f16)   # (128, 1540)
        aux16 = aux_sb.bitcast(f16)   # (1, 768)

        # Each dma_start costs ~610ns on its issuing sequencer and rings
        # process their FIFO in order, so: xt pieces own the sync ring (the
        # S phase streams them; first piece small so S starts early), xc
        # follows on the same ring (needed only at the output phase),
        # wpk/aux ride the scalar ring.
        # piece 0 gets the sync ring EXCLUSIVELY so its completion sem is
        # not queued behind sibling descriptors; everything else rides the
        # scalar ring (all still arriving ahead of consumption).
        nc.sync.dma_start(out=xt_sb[:, 0 : 2 * 257], in_=xt[:, 0 : 2 * 257])
        nc.scalar.dma_start(out=wpk_sb, in_=wpk)
        nc.scalar.dma_start(out=aux_sb, in_=aux)
        pc0 = 2
        for pc in [2, 4, 4, 6, 6]:
            psl = slice(pc0 * 257, (pc0 + pc) * 257)
            nc.scalar.dma_start(out=xt_sb[:, psl], in_=xt[:, psl])
            pc0 += pc
        for q in range(2):
            qsl = slice(q * 3072, (q + 1) * 3072)
            nc.scalar.dma_start(out=xc_sb[:, qsl], in_=xc[:, qsl])

        # S = X X^T accumulated over 24 pixel chunks; col 256 = sx (ones col)
        S0 = psS.tile([128, 512], f32, tag="s0", name="S0")[:, 0:257]
        S1 = psS.tile([128, 512], f32, tag="s1", name="S1")[:, 0:257]
        for i in range(NT):
            base = i * 257
            nc.tensor.matmul(
                S0, lhsT=xt_sb[:, base : base + 128],
                rhs=xt_sb[:, base : base + 257],
                start=(i == 0), stop=(i == NT - 1),
            )
            nc.tensor.matmul(
                S1, lhsT=xt_sb[:, base + 128 : base + 256],
                rhs=xt_sb[:, base : base + 257],
                start=(i == 0), stop=(i == NT - 1),
            )
        nc.scalar.copy(out=S0_sb, in_=S0)
        nc.vector.tensor_copy(S1_sb, S1)

        # SG[c, d'] = sum_c2 S[c2, c] g_w[d', c2]  (S symmetric) -- first on
        # the PE: it heads the critical chain S -> SG -> Mt -> R1 -> T2.
        SGp = psC.tile([128, 256], f32, tag="c", name="SGp")
        for h in range(2):
            hsl = slice(h * 128, (h + 1) * 128)
            nc.tensor.matmul(
                SGp[:, hsl], lhsT=S0_sb[:, hsl], rhs=wpk16[:, 128:256],
                start=True, stop=False,
            )
            nc.tensor.matmul(
                SGp[:, hsl], lhsT=S1_sb[:, hsl], rhs=wpk16[:, 384:512],
                start=False, stop=True,
            )
        # uv = [u | v] = sx^T [phi_w^T | g_w^T]
        uvp = psC.tile([128, 256], f32, tag="c", name="uvp")
        nc.tensor.matmul(
            uvp[0:1, :], lhsT=S0_sb[:, 256:257], rhs=wpk16[:, 0:256],
            start=True, stop=False,
        )
        nc.tensor.matmul(
            uvp[0:1, :], lhsT=S1_sb[:, 256:257], rhs=wpk16[:, 256:512],
            start=False, stop=True,
        )
        nc.scalar.copy(out=SG_sb[:, 0:128], in_=SGp[:, 0:128])
        nc.vector.tensor_copy(SG_sb[:, 128:256], SGp[:, 128:256])
        nc.vector.tensor_copy(uv_sb, uvp[0:1, :])

        # Mt[d', d] = M[d, d'] = SG^T phi_w^T + bg u^T + v bp^T + N bg bp^T
        # (all three rank-1 terms are pure matmuls on host-packed rows)
        Mtp = psC.tile([128, 128], f32, tag="c", name="Mtp")
        nc.tensor.matmul(
            Mtp, lhsT=SG_sb[:, 0:128], rhs=wpk16[:, 0:128],
            start=True, stop=False,
        )
        nc.tensor.matmul(
            Mtp, lhsT=SG_sb[:, 128:256], rhs=wpk16[:, 256:384],
            start=False, stop=False,
        )
        nc.tensor.matmul(
            Mtp, lhsT=aux16[0:1, 0:128], rhs=uv_sb[0:1, 0:128],
            start=False, stop=False,
        )
        nc.tensor.matmul(
            Mtp, lhsT=uv_sb[0:1, 128:256], rhs=aux16[0:1, 128:256],
            start=False, stop=False,
        )
        nc.tensor.matmul(
            Mtp, lhsT=aux16[0:1, 0:128], rhs=aux16[0:1, 512:640],
            start=False, stop=True,
        )
        nc.scalar.copy(out=Mt_sb[:, 0:64], in_=Mtp[:, 0:64])
        nc.vector.tensor_copy(Mt_sb[:, 64:128], Mtp[:, 64:128])

        # R1[d, c] = sum_d' Mt[d', d] w''[c, d']  (= V[c, d])
        R1p = psC.tile([128, 256], f32, tag="c", name="R1p")
        nc.tensor.matmul(R1p, lhsT=Mt_sb, rhs=wpk16[:, 512:768], start=True, stop=True)
        nc.scalar.copy(out=R1_sb[:, 0:128], in_=R1p[:, 0:128])
        nc.vector.tensor_copy(R1_sb[:, 128:256], R1p[:, 128:256])

        # T2 halves: T2[c', c] = theta_w^T R1 + I  (identity matmuls first:
        # they only need wpk + a free PSUM buffer, so they run early)
        T2ps = [
            psC.tile([128, 256], f32, tag="c", name=f"T2p{h}") for h in range(2)
        ]
        for h in range(2):
            nc.tensor.matmul(
                T2ps[h], lhsT=wpk16[:, 1024:1152],
                rhs=wpk16[:, 1024 + 256 * h : 1280 + 256 * h],
                start=True, stop=False,
            )
        for h in range(2):
            nc.tensor.matmul(
                T2ps[h], lhsT=wpk16[:, 768 + 128 * h : 896 + 128 * h], rhs=R1_sb,
                start=False, stop=True,
            )
        nc.scalar.copy(out=T2_sb[:, 0:256], in_=T2ps[0])
        nc.vector.tensor_copy(T2_sb[:, 256:512], T2ps[1])

        # z[c, n] = sum_c' T2[c', c] X[c', n] + b2[c]; fp16 out.
        # Per 512-pixel chunk j: two PSUM tiles (one per c-half), assembled
        # in parallel by ACT (half 0, bias via activation) and DVE (half 1,
        # tensor_scalar), one sync-issued DMA covering both c-halves of the
        # chunk via a strided DRAM AP. b2's tiny matmuls are emitted after
        # chunk 0's so they don't delay the first output matmul.
        for j in range(6):
            z_sb = zpool.tile([128, 1024], f16, tag="zs", name=f"z{j}")
            pzs = []
            for hc in range(2):
                pz = psZ.tile([128, 512], f32, tag="z", name=f"pz{j}_{hc}")
                pzs.append(pz)
                nc.tensor.matmul(
                    pz, lhsT=T2_sb[:, 128 * hc : 128 * hc + 128],
                    rhs=xc_sb[:, j * 1024 : j * 1024 + 512],
                    start=True, stop=False,
                )
                nc.tensor.matmul(
                    pz, lhsT=T2_sb[:, 256 + 128 * hc : 384 + 128 * hc],
                    rhs=xc_sb[:, j * 1024 + 512 : (j + 1) * 1024],
                    start=False, stop=True,
                )
            if j == 0:
                # b2[c] = sum_d R1[d, c] bt[d] + b'[c]
                b2p = psC.tile([128, 2], f32, tag="c", name="b2p")
                for h in range(2):
                    nc.tensor.matmul(
                        b2p[:, h : h + 1],
                        lhsT=R1_sb[:, h * 128 : (h + 1) * 128],
                        rhs=wpk16[:, 1536:1537], start=True, stop=False,
                    )
                    nc.tensor.matmul(
                        b2p[:, h : h + 1],
                        lhsT=aux16[0:1, 256 + 128 * h : 384 + 128 * h],
                        rhs=wpk16[0:1, 1024:1025], start=False, stop=True,
                    )
                nc.vector.tensor_copy(b2_sb, b2p)
            nc.scalar.activation(
                out=z_sb[:, 0:512], in_=pzs[0], func=AF.Identity,
                bias=b2_sb[:, 0:1], scale=1.0,
            )
            nc.vector.tensor_scalar_add(z_sb[:, 512:1024], pzs[1], b2_sb[:, 1:2])
            out_ap = bass.AP(
                tensor=out.tensor, offset=j * 512,
                ap=[[6144, 128], [3072, 2], [1, 512]],
            )
            nc.sync.dma_start(out=out_ap, in_=z_sb)

    nc.compile()
    return nc


def _get_nc():
    global _NC
    if _NC is None:
        _NC = _build_nc()
    return _NC


# test.py reads this after a traced run to get exec_time_ns
last_results = None


def _prep_inputs(inputs):
    import ml_dtypes

    f16 = ml_dtypes.bfloat16

    x = np.asarray(inputs["x"], dtype=np.float32)
    theta_w = np.asarray(inputs["theta_w"], np.float32)
    theta_b = np.asarray(inputs["theta_b"], np.float32)
    phi_w = np.asarray(inputs["phi_w"], np.float32)
    phi_b = np.asarray(inputs["phi_b"], np.float32)
    g_w = np.asarray(inputs["g_w"], np.float32)
    g_b = np.asarray(inputs["g_b"], np.float32)
    w_w = np.asarray(inputs["w_w"], np.float32)
    w_b = np.asarray(inputs["w_b"], np.float32)
    bn_gamma = np.asarray(inputs["bn_gamma"], np.float32)
    bn_beta = np.asarray(inputs["bn_beta"], np.float32)
    bn_mean = np.asarray(inputs["bn_mean"], np.float32)
    bn_var = np.asarray(inputs["bn_var"], np.float32)

    inv = bn_gamma / np.sqrt(bn_var + BN_EPS)
    bprime = inv * (w_b - bn_mean) + bn_beta                  # (C,)
    wpp = (w_w * inv[:, None]) / N                            # w'' (C, D)

    # wpk: per-partition packed weights (f16 cols)
    pgWT = np.concatenate(
        [phi_w.T[0:128], g_w.T[0:128], phi_w.T[128:256], g_w.T[128:256]],
        axis=1,
    )                                                         # (128, 512)
    Ipk = np.zeros((128, 512), np.float32)
    Ipk[np.arange(128), np.arange(128)] = 1.0                 # [I | 0]
    Ipk[np.arange(128), 384 + np.arange(128)] = 1.0           # [0 | I]
    btc = np.zeros((128, 4), np.float32)
    btc[:, 0] = theta_b
    wpk_f16 = np.concatenate(
        [pgWT, wpp.T, theta_w, Ipk, btc], axis=1
    ).astype(f16)                                             # (128, 1540)
    assert wpk_f16.shape == (128, 1540), wpk_f16.shape
    wpk = np.ascontiguousarray(wpk_f16).view(np.uint8).view(np.float32)      # (128, 770)

    aux_f16 = np.concatenate(
        [g_b, phi_b, bprime, N * phi_b, np.zeros(128, np.float32)]
    ).astype(f16)                                             # 768 f16 = 1536 B
    aux = aux_f16.view(np.uint8).view(np.float32)[None, :]                   # (1, 384)

    x16 = x.reshape(B, C, N).astype(f16)
    xt = np.ones((B, NT, 128, 257), f16)
    xt[:, :, :, 0:256] = x16.transpose(0, 2, 1).reshape(B, NT, 128, C)
    xt = xt.reshape(B, 128 * NT, 257)  # will re-chunk below
    xt = np.ascontiguousarray(
        xt.reshape(B, NT, 128, 257).transpose(0, 2, 1, 3).reshape(B, 128, NT * 257)
    )
    xc = np.ascontiguousarray(
        x16.reshape(B, 2, 128, 6, 512).transpose(0, 2, 3, 1, 4).reshape(B, 128, 6144)
    )
    return xt, xc, {"wpk": wpk, "aux": aux}


def kernel(**inputs):
    from concourse.bass_utils import run_bass_kernel_spmd

    global last_results

    xt, xc, shared = _prep_inputs(inputs)
    in_maps = [
        dict(shared, xt=np.ascontiguousarray(xt[b]), xc=np.ascontiguousarray(xc[b]))
        for b in range(B)
    ]

    nc = _get_nc()
    res = run_bass_kernel_spmd(nc, in_maps, list(range(N_CORES)))
    last_results = res

    outs = np.stack([res.results[b]["out"] for b in range(B)])  # (B, 128, 6144)
    z = outs.reshape(B, 128, 2, 3072).transpose(0, 2, 1, 3).reshape(B, C, N)
    return z.reshape(B, C, HH, WW).astype(np.float32)


# revision 55
# speedup vs baseline: 1.0169x; 1.0066x over previous
"""Non-local (dot-product attention) block kernel for Trainium2, 8 cores.

Reference math (per sample):
    t = theta_w @ X + bt 1^T            (D, N)
    p = phi_w @ X + bp 1^T              (D, N)
    g_x = (g_w @ X + bg 1^T)^T          (N, D)
    f = t^T p / N;  y = f g_x;  z = BN(w_w y^T) + x

Gram-form collapse (no softmax => pure matmul associativity). Since phi/g
only appear through M = p g, and t only through W2 = V theta_w:
    S  = X X^T                      (C, C)  Gram matrix
    sx = X 1                        (C,)    row sums (free via ones column)
    Mt = g_w S phi_w^T + bg(u+N bp)^T + v bp^T   u=phi_w sx, v=g_w sx
    R1 = Mt^T w''^T  (= V^T)        w'' = diag(inv) w_w / N
    T2 = theta_w^T R1 + I           (= W2^T + I: residual folded into weights)
    b2 = R1^T bt + b'
    z  = T2^T X + b2 1^T            one (C,C) x (C,N) output matmul

So the per-pixel work is just two C x C x N matmuls (Gram + output); the
whole projection/bias machinery shrinks to a ~10-matmul D-sized chain.

Device plan per core (data-parallel, one sample per core, no collectives):
  - Inputs: x uploaded twice (n-major xt for the Gram, c-major xc for the
    output matmul), fp16; weights byte-packed into wpk + a 1-partition aux.
  - S accumulates in PSUM over 24 pixel chunks streamed by DMA; a host-side
    ones column (257th) makes column 256 of S the row-sums sx.
  - Chain runs on tiny matmuls (rank-1 bias terms are K=1 matmuls over
    host-packed rows) with ACT/DVE splitting the PSUM->SBUF hops.
  - Output phase: 12 (128,512) matmul pairs; per-partition bias b2 applied
    during the PSUM->SBUF copy (ACT half 0 / DVE half 1); 6 strided bf16
    out-DMAs on the sync ring; host widens to fp32.
"""

import numpy as np

B, C, HH, WW = 8, 256, 96, 32
N = HH * WW          # 3072
D = 128              # inter_channels
BN_EPS = 1e-5
NT = N // 128        # 24 pixel chunks
N_CORES = 8
NWARM = 4            # dummy matmuls to lift the PE HAM clock gate

_NC = None


def _build_nc():
    from contextlib import ExitStack

    import concourse.bass as bass
    import concourse.bacc as bacc
    import concourse.tile as tile
    from concourse import mybir

    f32 = mybir.dt.float32
    f16 = mybir.dt.bfloat16
    AF = mybir.ActivationFunctionType

    nc = bacc.Bacc(
        "TRN2",
        target_bir_lowering=False,
        debug=False,
        num_devices=N_CORES,
    )

    # xt: 24 blocks of 257 cols: block i = x^T[128i:128(i+1), :] | ones
    xt = nc.dram_tensor("xt", [128, NT * 257], f16, kind="ExternalInput").ap()
    # xc: 6 blocks of 1024: block j = [x[0:128, 512j:512j+512] | x[128:256, ...]]
    xc = nc.dram_tensor("xc", [128, 6144], f16, kind="ExternalInput").ap()
    # wpk f16 cols: pgWT 512 | wT2 256 | thW 256 | Ipk 512 | bt 2 | pad 2
    wpk = nc.dram_tensor("wpk", [128, 770], f32, kind="ExternalInput").ap()
    # aux f16 cols: bg 128 | bp 128 | b'0 128 | b'1 128 ; f32 cols 256:384 = N*bp
    aux = nc.dram_tensor("aux", [1, 384], f32, kind="ExternalInput").ap()
    # out: [z[0:128, :] | z[128:256, :]] fp16
    out = nc.dram_tensor("out", [128, 6144], f16, kind="ExternalOutput").ap()

    with tile.TileContext(nc) as tc, ExitStack() as ctx:
        const = ctx.enter_context(tc.tile_pool(name="const", bufs=1))
        zpool = ctx.enter_context(tc.tile_pool(name="zpool", bufs=4))
        psS = ctx.enter_context(tc.tile_pool(name="psS", bufs=1, space="PSUM"))
        psC = ctx.enter_context(tc.tile_pool(name="psC", bufs=2, space="PSUM"))
        psZ = ctx.enter_context(tc.tile_pool(name="psZ", bufs=3, space="PSUM"))

        xt_sb = const.tile([128, NT * 257], f16)
        xc_sb = const.tile([128, 6144], f16)
        wpk_sb = const.tile([128, 770], f32)
        aux_sb = const.tile([1, 384], f32)
        S0_sb = const.tile([128, 257], f16)
        S1_sb = const.tile([128, 257], f16)
        uv_sb = const.tile([1, 256], f16)
        SG_sb = const.tile([128, 256], f16)
        Mt_sb = const.tile([128, 128], f16)
        R1_sb = const.tile([128, 256], f16)
        T2_sb = const.tile([128, 512], f16)
        b2_sb = const.tile([128, 2], f32)

        wpk16 = wpk_sb.bitcast(f16)   # (128, 1540)
        aux16 = aux_sb.bitcast(f16)   # (1, 768)

        # Each dma_start costs ~610ns on its issuing sequencer and rings
        # process their FIFO in order, so: xt pieces own the sync ring (the
        # S phase streams them; first piece small so S starts early), xc
        # follows on the same ring (needed only at the output phase),
        # wpk/aux ride the scalar ring.
        nc.scalar.dma_start(out=wpk_sb, in_=wpk)
        nc.scalar.dma_start(out=aux_sb, in_=aux)
        piece_chunks = [2, 2, 4, 4, 6, 6]
        pc0 = 0
        for pc in piece_chunks:
            psl = slice(pc0 * 257, (pc0 + pc) * 257)
            nc.sync.dma_start(out=xt_sb[:, psl], in_=xt[:, psl])
            pc0 += pc
        for q in range(2):
            qsl = slice(q * 3072, (q + 1) * 3072)
            nc.sync.dma_start(out=xc_sb[:, qsl], in_=xc[:, qsl])

        # S = X X^T accumulated over 24 pixel chunks; col 256 = sx (ones col)
        S0 = psS.tile([128, 512], f32, tag="s0", name="S0")[:, 0:257]
        S1 = psS.tile([128, 512], f32, tag="s1", name="S1")[:, 0:257]
        for i in range(NT):
            base = i * 257
            nc.tensor.matmul(
                S0, lhsT=xt_sb[:, base : base + 128],
                rhs=xt_sb[:, base : base + 257],
                start=(i == 0), stop=(i == NT - 1),
            )
            nc.tensor.matmul(
                S1, lhsT=xt_sb[:, base + 128 : base + 256],
                rhs=xt_sb[:, base : base + 257],
                start=(i == 0), stop=(i == NT - 1),
            )
        nc.scalar.copy(out=S0_sb, in_=S0)
        nc.vector.tensor_copy(S1_sb, S1)

        # SG[c, d'] = sum_c2 S[c2, c] g_w[d', c2]  (S symmetric) -- first on
        # the PE: it heads the critical chain S -> SG -> Mt -> R1 -> T2.
        SGp = psC.tile([128, 256], f32, tag="c", name="SGp")
        for h in range(2):
            hsl = slice(h * 128, (h + 1) * 128)
            nc.tensor.matmul(
                SGp[:, hsl], lhsT=S0_sb[:, hsl], rhs=wpk16[:, 128:256],
                start=True, stop=False,
            )
            nc.tensor.matmul(
                SGp[:, hsl], lhsT=S1_sb[:, hsl], rhs=wpk16[:, 384:512],
                start=False, stop=True,
            )
        # uv = [u | v] = sx^T [phi_w^T | g_w^T]
        uvp = psC.tile([128, 256], f32, tag="c", name="uvp")
        nc.tensor.matmul(
            uvp[0:1, :], lhsT=S0_sb[:, 256:257], rhs=wpk16[:, 0:256],
            start=True, stop=False,
        )
        nc.tensor.matmul(
            uvp[0:1, :], lhsT=S1_sb[:, 256:257], rhs=wpk16[:, 256:512],
            start=False, stop=True,
        )
        nc.scalar.copy(out=SG_sb[:, 0:128], in_=SGp[:, 0:128])
        nc.vector.tensor_copy(SG_sb[:, 128:256], SGp[:, 128:256])
        nc.vector.tensor_copy(uv_sb, uvp[0:1, :])

        # Mt[d', d] = M[d, d'] = SG^T phi_w^T + bg u^T + v bp^T + N bg bp^T
        # (all three rank-1 terms are pure matmuls on host-packed rows)
        Mtp = psC.tile([128, 128], f32, tag="c", name="Mtp")
        nc.tensor.matmul(
            Mtp, lhsT=SG_sb[:, 0:128], rhs=wpk16[:, 0:128],
            start=True, stop=False,
        )
        nc.tensor.matmul(
            Mtp, lhsT=SG_sb[:, 128:256], rhs=wpk16[:, 256:384],
            start=False, stop=False,
        )
        nc.tensor.matmul(
            Mtp, lhsT=aux16[0:1, 0:128], rhs=uv_sb[0:1, 0:128],
            start=False, stop=False,
        )
        nc.tensor.matmul(
            Mtp, lhsT=uv_sb[0:1, 128:256], rhs=aux16[0:1, 128:256],
            start=False, stop=False,
        )
        nc.tensor.matmul(
            Mtp, lhsT=aux16[0:1, 0:128], rhs=aux16[0:1, 512:640],
            start=False, stop=True,
        )
        nc.scalar.copy(out=Mt_sb[:, 0:64], in_=Mtp[:, 0:64])
        nc.vector.tensor_copy(Mt_sb[:, 64:128], Mtp[:, 64:128])

        # R1[d, c] = sum_d' Mt[d', d] w''[c, d']  (= V[c, d])
        R1p = psC.tile([128, 256], f32, tag="c", name="R1p")
        nc.tensor.matmul(R1p, lhsT=Mt_sb, rhs=wpk16[:, 512:768], start=True, stop=True)
        nc.scalar.copy(out=R1_sb[:, 0:128], in_=R1p[:, 0:128])
        nc.vector.tensor_copy(R1_sb[:, 128:256], R1p[:, 128:256])

        # T2 halves: T2[c', c] = theta_w^T R1 + I  (identity matmuls first:
        # they only need wpk + a free PSUM buffer, so they run early)
        T2ps = [
            psC.tile([128, 256], f32, tag="c", name=f"T2p{h}") for h in range(2)
        ]
        for h in range(2):
            nc.tensor.matmul(
                T2ps[h], lhsT=wpk16[:, 1024:1152],
                rhs=wpk16[:, 1024 + 256 * h : 1280 + 256 * h],
                start=True, stop=False,
            )
        for h in range(2):
            nc.tensor.matmul(
                T2ps[h], lhsT=wpk16[:, 768 + 128 * h : 896 + 128 * h], rhs=R1_sb,
                start=False, stop=True,
            )
        nc.scalar.copy(out=T2_sb[:, 0:256], in_=T2ps[0])
        nc.vector.tensor_copy(T2_sb[:, 256:512], T2ps[1])

        # z[c, n] = sum_c' T2[c', c] X[c', n] + b2[c]; fp16 out.
        # Per 512-pixel chunk j: two PSUM tiles (one per c-half), assembled
        # in parallel by ACT (half 0, bias via activation) and DVE (half 1,
        # tensor_scalar), one sync-issued DMA covering both c-halves of the
        # chunk via a strided DRAM AP. b2's tiny matmuls are emitted after
        # chunk 0's so they don't delay the first output matmul.
        for j in range(6):
            z_sb = zpool.tile([128, 1024], f16, tag="zs", name=f"z{j}")
            pzs = []
            for hc in range(2):
                pz = psZ.tile([128, 512], f32, tag="z", name=f"pz{j}_{hc}")
                pzs.append(pz)
                nc.tensor.matmul(
                    pz, lhsT=T2_sb[:, 128 * hc : 128 * hc + 128],
                    rhs=xc_sb[:, j * 1024 : j * 1024 + 512],
                    start=True, stop=False,
                )
                nc.tensor.matmul(
                    pz, lhsT=T2_sb[:, 256 + 128 * hc : 384 + 128 * hc],
                    rhs=xc_sb[:, j * 1024 + 512 : (j + 1) * 1024],
                    start=False, stop=True,
                )
            if j == 0:
                # b2[c] = sum_d R1[d, c] bt[d] + b'[c]
                b2p = psC.tile([128, 2], f32, tag="c", name="b2p")
                for h in range(2):
                    nc.tensor.matmul(
                        b2p[:, h : h + 1],
                        lhsT=R1_sb[:, h * 128 : (h + 1) * 128],
                        rhs=wpk16[:, 1536:1537], start=True, stop=False,
                    )
                    nc.tensor.matmul(
                        b2p[:, h : h + 1],
                        lhsT=aux16[0:1, 256 + 128 * h : 384 + 128 * h],
                        rhs=wpk16[0:1, 1024:1025], start=False, stop=True,
                    )
                nc.vector.tensor_copy(b2_sb, b2p)
            nc.scalar.activation(
                out=z_sb[:, 0:512], in_=pzs[0], func=AF.Identity,
                bias=b2_sb[:, 0:1], scale=1.0,
            )
            nc.vector.tensor_scalar_add(z_sb[:, 512:1024], pzs[1], b2_sb[:, 1:2])
            out_ap = bass.AP(
                tensor=out.tensor, offset=j * 512,
                ap=[[6144, 128], [3072, 2], [1, 512]],
            )
            nc.sync.dma_start(out=out_ap, in_=z_sb)

    nc.compile()
    return nc


def _get_nc():
    global _NC
    if _NC is None:
        _NC = _build_nc()
    return _NC


# test.py reads this after a traced run to get exec_time_ns
last_results = None


def _prep_inputs(inputs):
    import ml_dtypes

    f16 = ml_dtypes.bfloat16

    x = np.asarray(inputs["x"], dtype=np.float32)
    theta_w = np.asarray(inputs["theta_w"], np.float32)
    theta_b = np.asarray(inputs["theta_b"], np.float32)
    phi_w = np.asarray(inputs["phi_w"], np.float32)
    phi_b = np.asarray(inputs["phi_b"], np.float32)
    g_w = np.asarray(inputs["g_w"], np.float32)
    g_b = np.asarray(inputs["g_b"], np.float32)
    w_w = np.asarray(inputs["w_w"], np.float32)
    w_b = np.asarray(inputs["w_b"], np.float32)
    bn_gamma = np.asarray(inputs["bn_gamma"], np.float32)
    bn_beta = np.asarray(inputs["bn_beta"], np.float32)
    bn_mean = np.asarray(inputs["bn_mean"], np.float32)
    bn_var = np.asarray(inputs["bn_var"], np.float32)

    inv = bn_gamma / np.sqrt(bn_var + BN_EPS)
    bprime = inv * (w_b - bn_mean) + bn_beta                  # (C,)
    wpp = (w_w * inv[:, None]) / N                            # w'' (C, D)

    # wpk: per-partition packed weights (f16 cols)
    pgWT = np.concatenate(
        [phi_w.T[0:128], g_w.T[0:128], phi_w.T[128:256], g_w.T[128:256]],
        axis=1,
    )                                                         # (128, 512)
    Ipk = np.zeros((128, 512), np.float32)
    Ipk[np.arange(128), np.arange(128)] = 1.0                 # [I | 0]
    Ipk[np.arange(128), 384 + np.arange(128)] = 1.0           # [0 | I]
    btc = np.zeros((128, 4), np.float32)
    btc[:, 0] = theta_b
    wpk_f16 = np.concatenate(
        [pgWT, wpp.T, theta_w, Ipk, btc], axis=1
    ).astype(f16)                                             # (128, 1540)
    assert wpk_f16.shape == (128, 1540), wpk_f16.shape
    wpk = np.ascontiguousarray(wpk_f16).view(np.uint8).view(np.float32)      # (128, 770)

    aux_f16 = np.concatenate(
        [g_b, phi_b, bprime, N * phi_b, np.zeros(128, np.float32)]
    ).astype(f16)                                             # 768 f16 = 1536 B
    aux = aux_f16.view(np.uint8).view(np.float32)[None, :]                   # (1, 384)

    x16 = x.reshape(B, C, N).astype(f16)
    xt = np.ones((B, NT, 128, 257), f16)
    xt[:, :, :, 0:256] = x16.transpose(0, 2, 1).reshape(B, NT, 128, C)
    xt = xt.reshape(B, 128 * NT, 257)  # will re-chunk below
    xt = np.ascontiguousarray(
        xt.reshape(B, NT, 128, 257).transpose(0, 2, 1, 3).reshape(B, 128, NT * 257)
    )
    xc = np.ascontiguousarray(
        x16.reshape(B, 2, 128, 6, 512).transpose(0, 2, 3, 1, 4).reshape(B, 128, 6144)
    )
    return xt, xc, {"wpk": wpk, "aux": aux}


def kernel(**inputs):
    from concourse.bass_utils import run_bass_kernel_spmd

    global last_results

    xt, xc, shared = _prep_inputs(inputs)
    in_maps = [
        dict(shared, xt=np.ascontiguousarray(xt[b]), xc=np.ascontiguousarray(xc[b]))
        for b in range(B)
    ]

    nc = _get_nc()
    res = run_bass_kernel_spmd(nc, in_maps, list(range(N_CORES)))
    last_results = res

    outs = np.stack([res.results[b]["out"] for b in range(B)])  # (B, 128, 6144)
    z = outs.reshape(B, 128, 2, 3072).transpose(0, 2, 1, 3).reshape(B, C, N)
    return z.reshape(B, C, HH, WW).astype(np.float32)
